# revision 16
# baseline (speedup 1.0000x reference)
"""AttentionDecoder (2-layer residual LSTM + dot attention + vocab classifier)
on 8 Trainium2 NeuronCores.

Sharding:
  - Tensor-parallel shard of the LSTM gate matrices [4H,H] over the 4H dim:
    each core owns a 128-wide h-slice of all four gates, both layers; one
    AllGather of the new (h0, h1) slices (f32) per scan step.
  - Input projections x @ W_ih.T hoisted out of the scan.
  - Classifier [V,H] sharded over V, streamed m-tile-by-m-tile overlapped
    with the scan; attention batch-sharded, computed at the end in f32.
All matmuls bf16 except attention scores (f32); LSTM state stays f32.
"""

import numpy as np
from contextlib import ExitStack

B, T, S, H, V = 32, 64, 128, 1024, 32000
NCORES = 8
KT = H // 128            # 8 k-tiles over the contraction dim
HSH = H // NCORES        # 128-wide h-slice per core
GW = 4 * HSH             # 512 gate width per core; free layout [i|f|o|g]
VS = V // NCORES         # 4000 vocab shard
NT = 8                   # classifier n tiles
NW = VS // NT            # 500
MT = (B * T) // 128      # 16 classifier m tiles
BL = B // NCORES         # 4 local batches for attention


def _kp(ap):
    # [H, w] dram AP -> [128, KT, w] iterated (p, k, w)
    return ap.rearrange("(k p) w -> p k w", p=128)


def build_nc():
    import concourse.bacc as bacc
    import concourse.tile as tile
    from concourse import mybir
    from concourse.masks import make_identity
    from concourse._compat import get_trn_type

    f32 = mybir.dt.float32
    bf16 = mybir.dt.bfloat16
    AF = mybir.ActivationFunctionType
    ALU = mybir.AluOpType
    AXL = mybir.AxisListType

    nc = bacc.Bacc(get_trn_type() or "TRN2", target_bir_lowering=False, debug=False)

    # ------------- dram parameters (per-core values via in_maps) -----------
    EI, EO = "ExternalInput", "ExternalOutput"
    xt_raw = nc.dram_tensor("xt_raw", [H, B * T], f32, kind=EI)
    w_ih0t = nc.dram_tensor("w_ih0t", [H, GW], f32, kind=EI)
    w_hh0t = nc.dram_tensor("w_hh0t", [H, GW], f32, kind=EI)
    w_ih1t = nc.dram_tensor("w_ih1t", [H, GW], f32, kind=EI)
    w_hh1t = nc.dram_tensor("w_hh1t", [H, GW], f32, kind=EI)
    b0_row = nc.dram_tensor("b0_row", [2, GW], f32, kind=EI)
    b1_row = nc.dram_tensor("b1_row", [2, GW], f32, kind=EI)
    h0t_init = nc.dram_tensor("h0t_init", [H, B], f32, kind=EI)
    h1t_init = nc.dram_tensor("h1t_init", [H, B], f32, kind=EI)
    h1t_init_sl = nc.dram_tensor("h1t_init_sl", [HSH, B], f32, kind=EI)
    c_init = nc.dram_tensor("c_init", [2, B, HSH], f32, kind=EI)
    enct = nc.dram_tensor("enct", [B, H, S], f32, kind=EI)
    cls_wt = nc.dram_tensor("cls_wt", [H, VS], f32, kind=EI)
    cls_b_row = nc.dram_tensor("cls_b_row", [1, VS], f32, kind=EI)

    logits_out = nc.dram_tensor("logits_shard", [B, T, VS], f32, kind=EO)
    attn_out = nc.dram_tensor("attn_full", [B, S, T], f32, kind=EO)
    h_fin = nc.dram_tensor("h_final", [2, B, HSH], f32, kind=EO)
    c_fin = nc.dram_tensor("c_final", [2, B, HSH], f32, kind=EO)

    # internal dram
    xtr = nc.dram_tensor("xtr", [H, B * T], f32)        # relu(x), transposed
    outst = nc.dram_tensor("outst", [H, B * T], f32)    # 'last'^T, f32
    outst_bf = nc.dram_tensor("outst_bf", [H, B * T], bf16)
    xpj_dram = nc.dram_tensor("xpj_dram", [2, B * T, GW], bf16)

    rg = [list(range(NCORES))]

    with tile.TileContext(nc) as tc, ExitStack() as ctx:
        const = ctx.enter_context(tc.tile_pool(name="const", bufs=1))
        st = ctx.enter_context(tc.tile_pool(name="st", bufs=3))
        hbuf = ctx.enter_context(tc.tile_pool(name="hbuf", bufs=3))
        clsout = ctx.enter_context(tc.tile_pool(name="clsout", bufs=2))
        clsin = ctx.enter_context(tc.tile_pool(name="clsin", bufs=2))
        psum_g = ctx.enter_context(tc.tile_pool(name="psum_g", bufs=3, space="PSUM"))
        psum_tr = ctx.enter_context(tc.tile_pool(name="psum_tr", bufs=2, space="PSUM"))
        psum_m = ctx.enter_context(tc.tile_pool(name="psum_m", bufs=3, space="PSUM"))
        dram = ctx.enter_context(tc.tile_pool(name="dram", bufs=3, space="DRAM"))

        ident = const.tile([128, 128], f32, name="ident")
        make_identity(nc, ident[:])
        ones_row = const.tile([2, 128], f32, name="ones_row")
        nc.gpsimd.memset(ones_row[:], 1.0)
        ones_bf = const.tile([1, 128], bf16, name="ones_bf")
        nc.gpsimd.memset(ones_bf[:], 1.0)

        # ---- load scan weights + classifier weights (f32 -> bf16 sbuf) ----
        w_sb = {}
        wconst_cm = tc.tile_pool(name="wconst", bufs=1)
        wconst = wconst_cm.__enter__()
        with tc.tile_pool(name="wload", bufs=2) as wload:
            for name, drt in (("w0h", w_hh0t), ("w1i", w_ih1t), ("w1h", w_hh1t)):
                out = wconst.tile([128, KT, GW], bf16, name=name)
                for k in range(KT):
                    tmp = wload.tile([128, GW], f32, name="wtmp")
                    nc.sync.dma_start(tmp[:], _kp(drt[:])[:, k])
                    nc.vector.tensor_copy(out[:, k], tmp[:])
                w_sb[name] = out
            cls_wtb = wconst.tile([128, KT, VS], bf16, name="cls_wtb")
            for n in range(NT):
                for k in range(KT):
                    tmp = wload.tile([128, NW], f32, name="clstmp")
                    nc.sync.dma_start(
                        tmp[:], _kp(cls_wt[:])[:, k, n * NW:(n + 1) * NW]
                    )
                    nc.vector.tensor_copy(cls_wtb[:, k, n * NW:(n + 1) * NW], tmp[:])
            b0r = const.tile([2, GW], f32, name="b0r")
            nc.sync.dma_start(b0r[:], b0_row[:])
            b1r = const.tile([2, GW], f32, name="b1r")
            nc.sync.dma_start(b1r[:], b1_row[:])
            clsbr = wconst.tile([1, VS], bf16, name="clsbr")
            for n in range(NT):
                clsbr_f = wload.tile([1, NW], f32, name="clsbr_f")
                nc.sync.dma_start(clsbr_f[:], cls_b_row[:, n * NW:(n + 1) * NW])
                nc.vector.tensor_copy(clsbr[:, n * NW:(n + 1) * NW], clsbr_f[:])

        # ---- relu(x) -> xtr, and hoisted input projections ----------------
        with tc.tile_pool(name="xload", bufs=2) as xload:
            for m in range(MT):
                xc = xload.tile([128, KT, 128], f32, name="xstage")
                nc.sync.dma_start(
                    xc[:], _kp(xt_raw[:])[:, :, m * 128:(m + 1) * 128]
                )
                nc.scalar.activation(xc[:], xc[:], AF.Relu)
                nc.sync.dma_start(
                    _kp(xtr[:])[:, :, m * 128:(m + 1) * 128], xc[:]
                )
            for l, wdr in ((0, w_ih0t), (1, w_ih1t)):
                brow = b0r if l == 0 else b1r
                wb = xload.tile([128, KT, GW], bf16, name="wib", bufs=1)
                for k in range(KT):
                    tmp = xload.tile([128, GW], f32, name="wtmp2")
                    nc.sync.dma_start(tmp[:], _kp(wdr[:])[:, k])
                    nc.vector.tensor_copy(wb[:, k], tmp[:])
                for m in range(MT):
                    xmf = xload.tile([128, KT, 128], f32, name="xstage")
                    nc.sync.dma_start(
                        xmf[:], _kp(xtr[:])[:, :, m * 128:(m + 1) * 128]
                    )
                    xmb = xload.tile([128, KT, 128], bf16, name="xmb")
                    nc.vector.tensor_copy(xmb[:], xmf[:])
                    ps = psum_m.tile([128, 512], f32, name="pm")
                    for k in range(KT):
                        nc.tensor.matmul(
                            ps[:, :GW], xmb[:, k], wb[:, k],
                            start=(k == 0), stop=False,
                        )
                    nc.tensor.matmul(
                        ps[:, :GW], ones_row[:], brow[:], start=False, stop=True
                    )
                    xpb = xload.tile([128, GW], bf16, name="xpb")
                    nc.vector.tensor_copy(xpb[:], ps[:, :GW])
                    nc.sync.dma_start(
                        xpj_dram[l, m * 128:(m + 1) * 128, :], xpb[:]
                    )

        # ---- initial state ------------------------------------------------
        h0t_cur = hbuf.tile([128, KT, B], f32, name="h0t_f")
        nc.sync.dma_start(h0t_cur[:], _kp(h0t_init[:]))
        h1t_cur = hbuf.tile([128, KT, B], f32, name="h1t_f")
        nc.sync.dma_start(h1t_cur[:], _kp(h1t_init[:]))
        h0t_b = hbuf.tile([128, KT, B], bf16, name="h0t_b")
        nc.vector.tensor_copy(h0t_b[:], h0t_cur[:])
        h1t_b = hbuf.tile([128, KT, B], bf16, name="h1t_b")
        nc.vector.tensor_copy(h1t_b[:], h1t_cur[:])

        c_s = []
        for l in range(2):
            cs = st.tile([B, HSH], f32, name=f"c{l}s")
            nc.sync.dma_start(cs[:], c_init[l])
            c_s.append(cs)

        contrib = st.tile([128, 2, B], f32, name="contrib")
        h1isl = st.tile([128, B], f32, name="h1isl")
        nc.sync.dma_start(h1isl[:], h1t_init_sl[:])
        nc.vector.tensor_copy(contrib[:, 1], h1isl[:])

        def lstm_cell(gates_ps, xpj_sl, c_old, lname, extra_ps=None):
            """gates_ps [B,GW] psum; xpj_sl [B,GW] bf16; free layout
            [i|f|o|g]*HSH.  extra_ps: optional second psum operand (HW allows
            only one PSUM read per DVE op, so it is added separately)."""
            g = st.tile([B, GW], f32, name=f"g{lname}", bufs=2)
            nc.vector.tensor_add(g[:], gates_ps[:], xpj_sl)
            if extra_ps is not None:
                nc.vector.tensor_add(g[:], g[:], extra_ps[:])
            sact = st.tile([B, 3 * HSH], f32, name=f"s{lname}", bufs=2)
            nc.scalar.activation(sact[:], g[:, : 3 * HSH], AF.Sigmoid)
            gact = st.tile([B, HSH], f32, name=f"t{lname}", bufs=2)
            nc.scalar.activation(gact[:], g[:, 3 * HSH:], AF.Tanh)
            t1 = st.tile([B, HSH], f32, name=f"t1{lname}", bufs=2)
            nc.vector.tensor_mul(t1[:], sact[:, :HSH], gact[:])
            c_new = st.tile([B, HSH], f32, name=f"c{lname}n")
            nc.vector.tensor_mul(c_new[:], c_old[:], sact[:, HSH:2 * HSH])
            nc.vector.tensor_add(c_new[:], c_new[:], t1[:])
            ct = st.tile([B, HSH], f32, name=f"ct{lname}", bufs=2)
            nc.scalar.activation(ct[:], c_new[:], AF.Tanh)
            h_new = st.tile([B, HSH], f32, name=f"h{lname}n", bufs=2)
            nc.vector.tensor_mul(h_new[:], sact[:, 2 * HSH:], ct[:])
            return h_new, c_new

        cls_lhs = {}

        def emit_cls_tile(m, n):
            if n == 0:
                lhsb = clsin.tile([128, KT, 128], bf16, name="lhsb")
                nc.sync.dma_start(
                    lhsb[:], _kp(outst_bf[:])[:, :, m * 128:(m + 1) * 128]
                )
                cls_lhs[m] = lhsb
            lhsb = cls_lhs[m]
            nsl = slice(n * NW, (n + 1) * NW)
            ps = psum_m.tile([128, 512], f32, name="pm")
            for k in range(KT):
                nc.tensor.matmul(
                    ps[:, :NW], lhsb[:, k], cls_wtb[:, k, nsl],
                    start=(k == 0), stop=False,
                )
            nc.tensor.matmul(
                ps[:, :NW], ones_bf[:], clsbr[:, nsl], start=False, stop=True
            )
            osb = clsout.tile([128, NW], f32, name="osb")
            nc.vector.tensor_copy(osb[:], ps[:, :NW])
            dst = logits_out[:].rearrange("b (mt st) v -> mt st b v", st=4)
            nc.sync.dma_start(dst[m, :, :, nsl], osb[:])

        # ---- the scan -----------------------------------------------------
        # round r: mm0 -> h0_r slice (step r); mm1 -> h1_{r-1} slice
        # (step r-1); C_r = AllGather(h0_r, h1_{r-1}) slices, f32.
        # lastT for step r-2 also computed in round r.
        cc_out_prev = None
        h_fin_t = [None, None]
        c_fin_t = [None, None]
        cls_emitted = 0

        for r in range(T + 2):
            h0t_prev = h0t_cur
            if r >= 1:
                src = cc_out_prev[:].rearrange(
                    "(k l p) c -> p k l c", k=KT, l=2
                )
                h0t_cur = hbuf.tile([128, KT, B], f32, name="h0t_f")
                nc.sync.dma_start(h0t_cur[:], src[:, :, 0])
                h1t_cur = hbuf.tile([128, KT, B], f32, name="h1t_f")
                nc.scalar.dma_start(h1t_cur[:], src[:, :, 1])
                if r <= T:
                    h0t_b = hbuf.tile([128, KT, B], bf16, name="h0t_b")
                    nc.vector.tensor_copy(h0t_b[:], h0t_cur[:])
                    h1t_b = hbuf.tile([128, KT, B], bf16, name="h1t_b")
                    nc.vector.tensor_copy(h1t_b[:], h1t_cur[:])

            if r >= 2:
                # lastT for step t = r-2 (x_t + h0_t + h1_t), f32
                t_step = r - 2
                csl = slice(t_step * B, (t_step + 1) * B)
                xtc = st.tile([128, KT, B], f32, name="xtc")
                nc.sync.dma_start(xtc[:], _kp(xtr[:])[:, :, csl])
                lastt = st.tile([128, KT, B], f32, name="lastt")
                nc.vector.tensor_add(lastt[:], h0t_prev[:], h1t_cur[:])
                nc.vector.tensor_add(lastt[:], lastt[:], xtc[:])
                nc.sync.dma_start(_kp(outst[:])[:, :, csl], lastt[:])
                lastb = st.tile([128, KT, B], bf16, name="lastb")
                nc.vector.tensor_copy(lastb[:], lastt[:])
                nc.sync.dma_start(_kp(outst_bf[:])[:, :, csl], lastb[:])

            if r <= T:
                new_contrib = st.tile([128, 2, B], f32, name="contrib")

            # col-packed scan matmuls: 3 concurrent M=32 col groups
            if r <= T:
                psg = psum_g.tile([128, GW], f32, name="psg")
                for k in range(KT):
                    fl = dict(start=(k == 0), stop=(k == KT - 1),
                              skip_group_check=True)
                    if r < T:
                        nc.tensor.matmul(
                            psg[0:B], h0t_b[:, k], w_sb["w0h"][:, k],
                            tile_position=(0, 0), **fl,
                        )
                    if r >= 1:
                        nc.tensor.matmul(
                            psg[B:2 * B], h0t_b[:, k], w_sb["w1i"][:, k],
                            tile_position=(0, 32), **fl,
                        )
                        nc.tensor.matmul(
                            psg[2 * B:3 * B], h1t_b[:, k], w_sb["w1h"][:, k],
                            tile_position=(0, 64), **fl,
                        )

            if r < T:
                xp0 = st.tile([B, GW], bf16, name="xp0s", bufs=2)
                nc.sync.dma_start(xp0[:], xpj_dram[0, r * B:(r + 1) * B, :])
                h0_new, c0_new = lstm_cell(psg[0:B], xp0[:], c_s[0], "0")
                c_s[0] = c0_new
                if r == T - 1:
                    h_fin_t[0], c_fin_t[0] = h0_new, c0_new
                bt0 = st.tile([B, HSH], f32, name="bt0", bufs=2)
                nc.vector.transpose(bt0[:], h0_new[:])
                for i in range(HSH // B):
                    nc.gpsimd.tensor_copy(
                        new_contrib[B * i:B * (i + 1), 0],
                        bt0[:, B * i:B * (i + 1)],
                    )
            elif r == T:
                nc.vector.tensor_copy(new_contrib[:, 0], contrib[:, 0])

            if 1 <= r <= T:
                xp1 = st.tile([B, GW], bf16, name="xp1s", bufs=2)
                nc.sync.dma_start(
                    xp1[:], xpj_dram[1, (r - 1) * B:r * B, :]
                )
                h1_new, c1_new = lstm_cell(
                    psg[B:2 * B], xp1[:], c_s[1], "1",
                    extra_ps=psg[2 * B:3 * B],
                )
                c_s[1] = c1_new
                if r == T:
                    h_fin_t[1], c_fin_t[1] = h1_new, c1_new
                bt1 = st.tile([B, HSH], f32, name="bt1", bufs=2)
                nc.vector.transpose(bt1[:], h1_new[:])
                for i in range(HSH // B):
                    nc.gpsimd.tensor_copy(
                        new_contrib[B * i:B * (i + 1), 1],
                        bt1[:, B * i:B * (i + 1)],
                    )
            elif r == 0:
                nc.vector.tensor_copy(new_contrib[:, 1], contrib[:, 1])

            if r <= T:
                contrib = new_contrib
                cc_in = dram.tile([2 * 128, B], f32, name="cc_in")
                nc.gpsimd.dma_start(
                    cc_in[:].rearrange("(l p) c -> p l c", p=128), contrib[:]
                )
                cc_out = dram.tile([NCORES * 2 * 128, B], f32, name="cc_out")
                nc.gpsimd.collective_compute(
                    "AllGather",
                    ALU.bypass,
                    replica_groups=rg,
                    ins=[cc_in[:].opt()],
                    outs=[cc_out[:].opt()],
                )
                cc_out_prev = cc_out

            budget = 2
            while budget > 0 and cls_emitted < MT * NT:
                m, n = divmod(cls_emitted, NT)
                if r < 4 * m + 5 and r < T + 1:
                    break
                emit_cls_tile(m, n)
                cls_emitted += 1
                budget -= 1
            if r == T + 1:
                while cls_emitted < MT * NT:
                    m, n = divmod(cls_emitted, NT)
                    emit_cls_tile(m, n)
                    cls_emitted += 1

        # ---- final states -------------------------------------------------
        hc_sb = st.tile([B, 2, HSH], f32, name="hc_sb", bufs=1)
        nc.vector.tensor_copy(hc_sb[:, 0], h_fin_t[0][:])
        nc.vector.tensor_copy(hc_sb[:, 1], h_fin_t[1][:])
        nc.sync.dma_start(h_fin[:].rearrange("l b h -> b l h"), hc_sb[:])
        cc_sb = st.tile([B, 2, HSH], f32, name="cc_sb", bufs=1)
        nc.vector.tensor_copy(cc_sb[:, 0], c_fin_t[0][:])
        nc.vector.tensor_copy(cc_sb[:, 1], c_fin_t[1][:])
        nc.sync.dma_start(c_fin[:].rearrange("l b h -> b l h"), cc_sb[:])

        # ---- attention (f32, replicated; host keeps core 0's copy) --------
        wconst_cm.__exit__(None, None, None)

        with tc.tile_pool(name="att", bufs=2) as att:
            outs_all = att.tile([128, KT, T, B], f32, name="outs_all", bufs=1)
            nc.sync.dma_start(
                outs_all[:],
                outst[:].rearrange("(k p) (t b) -> p k t b", p=128, b=B),
            )
            for b in range(B):
                encb = att.tile([128, KT, S], f32, name="encb")
                nc.sync.dma_start(encb[:], _kp(enct[b]))
                pssc = psum_m.tile([128, 512], f32, name="pm")
                for k in range(KT):
                    nc.tensor.matmul(
                        pssc[0:T, 0:S], outs_all[:, k, :, b], encb[:, k],
                        start=(k == 0), stop=(k == KT - 1),
                    )
                nmax = att.tile([T, 1], f32, name="nmax")
                nc.vector.tensor_reduce(
                    nmax[:], pssc[0:T, 0:S], axis=AXL.X, op=ALU.max, negate=True
                )
                probs = att.tile([T, S], f32, name="probs")
                sums = att.tile([T, 1], f32, name="sums")
                nc.scalar.activation(
                    probs[:], pssc[0:T, 0:S], AF.Exp,
                    bias=nmax[:], scale=1.0, accum_out=sums[:],
                )
                rec = att.tile([T, 1], f32, name="rec")
                nc.vector.reciprocal(rec[:], sums[:])
                nc.vector.tensor_scalar_mul(probs[:], probs[:], rec[:])
                pst = psum_m.tile([128, 512], f32, name="pm")
                nc.tensor.transpose(pst[0:S, 0:T], probs[:], ident[0:T, 0:T])
                asb = att.tile([128, T], f32, name="asb")
                nc.vector.tensor_copy(asb[:], pst[0:S, 0:T])
                nc.sync.dma_start(attn_out[b], asb[:])

    nc.compile()
    return nc


def prep_in_maps(inputs):
    dec = np.asarray(inputs["dec_input"])
    h0 = np.asarray(inputs["h0"], np.float32)
    c0 = np.asarray(inputs["c0"], np.float32)
    enc = np.asarray(inputs["encoder_outputs"], np.float32)
    emb = np.asarray(inputs["emb"], np.float32)
    W_ih = np.asarray(inputs["W_ih"], np.float32)
    W_hh = np.asarray(inputs["W_hh"], np.float32)
    b_ih = np.asarray(inputs["b_ih"], np.float32)
    b_hh = np.asarray(inputs["b_hh"], np.float32)
    cls_W = np.asarray(inputs["cls_W"], np.float32)
    cls_b = np.asarray(inputs["cls_b"], np.float32)

    # xT columns ordered c = t*B + b
    idx = dec.T.reshape(-1)
    xt_raw = np.ascontiguousarray(emb[idx].T)
    enct_full = np.ascontiguousarray(enc.transpose(0, 2, 1))

    C = np.ascontiguousarray
    in_maps = []
    for k in range(NCORES):
        hs = slice(HSH * k, HSH * (k + 1))
        rows = np.concatenate([
            np.arange(0 * H, 1 * H)[hs],   # i
            np.arange(1 * H, 2 * H)[hs],   # f
            np.arange(3 * H, 4 * H)[hs],   # o
            np.arange(2 * H, 3 * H)[hs],   # g
        ])
        vs = slice(VS * k, VS * (k + 1))
        in_maps.append({
            "xt_raw": xt_raw,
            "w_ih0t": C(W_ih[0][rows].T),
            "w_hh0t": C(W_hh[0][rows].T),
            "w_ih1t": C(W_ih[1][rows].T),
            "w_hh1t": C(W_hh[1][rows].T),
            "b0_row": C(np.stack([b_ih[0][rows], b_hh[0][rows]])),
            "b1_row": C(np.stack([b_ih[1][rows], b_hh[1][rows]])),
            "h0t_init": C(h0[0].T),
            "h1t_init": C(h0[1].T),
            "h1t_init_sl": C(h0[1][:, hs].T),
            "c_init": C(c0[:, :, hs]),
            "enct": enct_full,
            "cls_wt": C(cls_W[vs].T),
            "cls_b_row": C(cls_b[vs][None, :]),
        })
    return in_maps


def assemble(results):
    logits = np.concatenate([r["logits_shard"] for r in results], axis=2)
    hT = np.concatenate([r["h_final"] for r in results], axis=2)
    cT = np.concatenate([r["c_final"] for r in results], axis=2)
    attn = results[0]["attn_full"]
    return logits, hT, cT, attn


def run_on_hw(nc, in_maps, trace=False):
    from concourse import bass_utils
    return bass_utils.run_bass_kernel_spmd(
        nc, in_maps, core_ids=list(range(NCORES)), trace=trace
    )


def kernel(**inputs):
    in_maps = prep_in_maps(inputs)
    nc = build_nc()
    res = run_on_hw(nc, in_maps, trace=False)
    return assemble(res.results)


# revision 17
# speedup vs baseline: 1.1037x; 1.1037x over previous
"""AttentionDecoder (2-layer residual LSTM + dot attention + vocab classifier)
on 8 Trainium2 NeuronCores.

Sharding:
  - Tensor-parallel shard of the LSTM gate matrices [4H,H] over the 4H dim:
    each core owns a 128-wide h-slice of all four gates, both layers; one
    AllGather of the new (h0, h1) slices (f32) per scan step.
  - Input projections x @ W_ih.T hoisted out of the scan.
  - Classifier [V,H] sharded over V, streamed m-tile-by-m-tile overlapped
    with the scan; attention batch-sharded, computed at the end in f32.
All matmuls bf16 except attention scores (f32); LSTM state stays f32.
"""

import numpy as np
from contextlib import ExitStack

B, T, S, H, V = 32, 64, 128, 1024, 32000
NCORES = 8
KT = H // 128            # 8 k-tiles over the contraction dim
HSH = H // NCORES        # 128-wide h-slice per core
GW = 4 * HSH             # 512 gate width per core; free layout [i|f|o|g]
VS = V // NCORES         # 4000 vocab shard
NT = 8                   # classifier n tiles
NW = VS // NT            # 500
MT = (B * T) // 128      # 16 classifier m tiles
BL = B // NCORES         # 4 local batches for attention


def _kp(ap):
    # [H, w] dram AP -> [128, KT, w] iterated (p, k, w)
    return ap.rearrange("(k p) w -> p k w", p=128)


def build_nc():
    import concourse.bacc as bacc
    import concourse.tile as tile
    from concourse import mybir
    from concourse.masks import make_identity
    from concourse._compat import get_trn_type

    f32 = mybir.dt.float32
    bf16 = mybir.dt.bfloat16
    AF = mybir.ActivationFunctionType
    ALU = mybir.AluOpType
    AXL = mybir.AxisListType

    nc = bacc.Bacc(get_trn_type() or "TRN2", target_bir_lowering=False, debug=False)

    # ------------- dram parameters (per-core values via in_maps) -----------
    EI, EO = "ExternalInput", "ExternalOutput"
    xt_raw = nc.dram_tensor("xt_raw", [H, B * T], f32, kind=EI)
    w_ih0t = nc.dram_tensor("w_ih0t", [H, GW], f32, kind=EI)
    w_hh0t = nc.dram_tensor("w_hh0t", [H, GW], f32, kind=EI)
    w_ih1t = nc.dram_tensor("w_ih1t", [H, GW], f32, kind=EI)
    w_hh1t = nc.dram_tensor("w_hh1t", [H, GW], f32, kind=EI)
    b0_row = nc.dram_tensor("b0_row", [2, GW], f32, kind=EI)
    b1_row = nc.dram_tensor("b1_row", [2, GW], f32, kind=EI)
    h0t_init = nc.dram_tensor("h0t_init", [H, B], f32, kind=EI)
    h1t_init = nc.dram_tensor("h1t_init", [H, B], f32, kind=EI)
    h1t_init_sl = nc.dram_tensor("h1t_init_sl", [HSH, B], f32, kind=EI)
    c_init = nc.dram_tensor("c_init", [2, B, HSH], f32, kind=EI)
    enct = nc.dram_tensor("enct", [B, H, S], f32, kind=EI)
    cls_wt = nc.dram_tensor("cls_wt", [H, VS], f32, kind=EI)
    cls_b_row = nc.dram_tensor("cls_b_row", [1, VS], f32, kind=EI)

    logits_out = nc.dram_tensor("logits_shard", [B, T, VS], f32, kind=EO)
    attn_out = nc.dram_tensor("attn_full", [B, S, T], f32, kind=EO)
    h_fin = nc.dram_tensor("h_final", [2, B, HSH], f32, kind=EO)
    c_fin = nc.dram_tensor("c_final", [2, B, HSH], f32, kind=EO)

    # internal dram
    xtr = nc.dram_tensor("xtr", [H, B * T], f32)        # relu(x), transposed
    outst = nc.dram_tensor("outst", [H, B * T], f32)    # 'last'^T, f32
    outst_bf = nc.dram_tensor("outst_bf", [H, B * T], bf16)
    xpj_dram = nc.dram_tensor("xpj_dram", [2, B * T, GW], bf16)

    rg = [list(range(NCORES))]

    with tile.TileContext(nc) as tc, ExitStack() as ctx:
        const = ctx.enter_context(tc.tile_pool(name="const", bufs=1))
        st = ctx.enter_context(tc.tile_pool(name="st", bufs=3))
        hbuf = ctx.enter_context(tc.tile_pool(name="hbuf", bufs=3))
        clsout = ctx.enter_context(tc.tile_pool(name="clsout", bufs=2))
        clsin = ctx.enter_context(tc.tile_pool(name="clsin", bufs=2))
        psum_g = ctx.enter_context(tc.tile_pool(name="psum_g", bufs=4, space="PSUM"))
        psum_m = ctx.enter_context(tc.tile_pool(name="psum_m", bufs=3, space="PSUM"))
        dram = ctx.enter_context(tc.tile_pool(name="dram", bufs=3, space="DRAM"))

        ident = const.tile([128, 128], f32, name="ident")
        make_identity(nc, ident[:])
        ones_row = const.tile([2, 128], f32, name="ones_row")
        nc.gpsimd.memset(ones_row[:], 1.0)
        ones_bf = const.tile([1, 128], bf16, name="ones_bf")
        nc.gpsimd.memset(ones_bf[:], 1.0)

        # ---- load scan weights + classifier weights (f32 -> bf16 sbuf) ----
        w_sb = {}
        wconst_cm = tc.tile_pool(name="wconst", bufs=1)
        wconst = wconst_cm.__enter__()
        with tc.tile_pool(name="wload", bufs=2) as wload:
            for name, drt in (("w0h", w_hh0t), ("w1i", w_ih1t), ("w1h", w_hh1t)):
                out = wconst.tile([128, KT, GW], bf16, name=name)
                for k in range(KT):
                    tmp = wload.tile([128, GW], f32, name="wtmp")
                    nc.sync.dma_start(tmp[:], _kp(drt[:])[:, k])
                    nc.vector.tensor_copy(out[:, k], tmp[:])
                w_sb[name] = out
            cls_wtb = wconst.tile([128, KT, VS], bf16, name="cls_wtb")
            for n in range(NT):
                for k in range(KT):
                    tmp = wload.tile([128, NW], f32, name="clstmp")
                    nc.sync.dma_start(
                        tmp[:], _kp(cls_wt[:])[:, k, n * NW:(n + 1) * NW]
                    )
                    nc.vector.tensor_copy(cls_wtb[:, k, n * NW:(n + 1) * NW], tmp[:])
            b0r = const.tile([2, GW], f32, name="b0r")
            nc.sync.dma_start(b0r[:], b0_row[:])
            b1r = const.tile([2, GW], f32, name="b1r")
            nc.sync.dma_start(b1r[:], b1_row[:])
            clsbr = wconst.tile([1, VS], bf16, name="clsbr")
            for n in range(NT):
                clsbr_f = wload.tile([1, NW], f32, name="clsbr_f")
                nc.sync.dma_start(clsbr_f[:], cls_b_row[:, n * NW:(n + 1) * NW])
                nc.vector.tensor_copy(clsbr[:, n * NW:(n + 1) * NW], clsbr_f[:])

        # ---- relu(x) -> xtr, and hoisted input projections ----------------
        with tc.tile_pool(name="xload", bufs=2) as xload:
            for m in range(MT):
                xc = xload.tile([128, KT, 128], f32, name="xstage")
                nc.sync.dma_start(
                    xc[:], _kp(xt_raw[:])[:, :, m * 128:(m + 1) * 128]
                )
                nc.scalar.activation(xc[:], xc[:], AF.Relu)
                nc.sync.dma_start(
                    _kp(xtr[:])[:, :, m * 128:(m + 1) * 128], xc[:]
                )
            for l, wdr in ((0, w_ih0t), (1, w_ih1t)):
                brow = b0r if l == 0 else b1r
                wb = xload.tile([128, KT, GW], bf16, name="wib", bufs=1)
                for k in range(KT):
                    tmp = xload.tile([128, GW], f32, name="wtmp2")
                    nc.sync.dma_start(tmp[:], _kp(wdr[:])[:, k])
                    nc.vector.tensor_copy(wb[:, k], tmp[:])
                for m in range(MT):
                    xmf = xload.tile([128, KT, 128], f32, name="xstage")
                    nc.sync.dma_start(
                        xmf[:], _kp(xtr[:])[:, :, m * 128:(m + 1) * 128]
                    )
                    xmb = xload.tile([128, KT, 128], bf16, name="xmb")
                    nc.vector.tensor_copy(xmb[:], xmf[:])
                    ps = psum_m.tile([128, 512], f32, name="pm")
                    for k in range(KT):
                        nc.tensor.matmul(
                            ps[:, :GW], xmb[:, k], wb[:, k],
                            start=(k == 0), stop=False,
                        )
                    nc.tensor.matmul(
                        ps[:, :GW], ones_row[:], brow[:], start=False, stop=True
                    )
                    xpb = xload.tile([128, GW], bf16, name="xpb")
                    nc.vector.tensor_copy(xpb[:], ps[:, :GW])
                    nc.sync.dma_start(
                        xpj_dram[l, m * 128:(m + 1) * 128, :], xpb[:]
                    )

        # ---- initial state ------------------------------------------------
        h0t_cur = hbuf.tile([128, KT, B], f32, name="h0t_f")
        nc.sync.dma_start(h0t_cur[:], _kp(h0t_init[:]))
        h1t_cur = hbuf.tile([128, KT, B], f32, name="h1t_f")
        nc.sync.dma_start(h1t_cur[:], _kp(h1t_init[:]))
        h0t_b = hbuf.tile([128, KT, B], bf16, name="h0t_b")
        nc.vector.tensor_copy(h0t_b[:], h0t_cur[:])
        h1t_b = hbuf.tile([128, KT, B], bf16, name="h1t_b")
        nc.vector.tensor_copy(h1t_b[:], h1t_cur[:])

        c_s = []
        for l in range(2):
            cs = st.tile([B, HSH], f32, name=f"c{l}s")
            nc.sync.dma_start(cs[:], c_init[l])
            c_s.append(cs)

        h1isl = st.tile([128, B], f32, name="h1isl")
        nc.sync.dma_start(h1isl[:], h1t_init_sl[:])
        bt0_prev = None

        def lstm_cell(gates_ps, xpj_sl, c_old, lname, extra_ps=None):
            """gates_ps [B,GW] psum; xpj_sl [B,GW] bf16; free layout
            [i|f|o|g]*HSH.  extra_ps: optional second psum operand (HW allows
            only one PSUM read per DVE op, so it is added separately)."""
            g = st.tile([B, GW], f32, name=f"g{lname}", bufs=2)
            nc.vector.tensor_add(g[:], gates_ps[:], xpj_sl)
            if extra_ps is not None:
                nc.vector.tensor_add(g[:], g[:], extra_ps[:])
            sact = st.tile([B, 3 * HSH], f32, name=f"s{lname}", bufs=2)
            nc.scalar.activation(sact[:], g[:, : 3 * HSH], AF.Sigmoid)
            gact = st.tile([B, HSH], f32, name=f"t{lname}", bufs=2)
            nc.scalar.activation(gact[:], g[:, 3 * HSH:], AF.Tanh)
            t1 = st.tile([B, HSH], f32, name=f"t1{lname}", bufs=2)
            nc.vector.tensor_mul(t1[:], sact[:, :HSH], gact[:])
            c_new = st.tile([B, HSH], f32, name=f"c{lname}n")
            nc.vector.tensor_mul(c_new[:], c_old[:], sact[:, HSH:2 * HSH])
            nc.vector.tensor_add(c_new[:], c_new[:], t1[:])
            ct = st.tile([B, HSH], f32, name=f"ct{lname}", bufs=2)
            nc.scalar.activation(ct[:], c_new[:], AF.Tanh)
            h_new = st.tile([B, HSH], f32, name=f"h{lname}n", bufs=2)
            nc.vector.tensor_mul(h_new[:], sact[:, 2 * HSH:], ct[:])
            return h_new, c_new

        cls_lhs = {}

        def emit_cls_tile(m, n):
            if n == 0:
                lhsb = clsin.tile([128, KT, 128], bf16, name="lhsb")
                nc.gpsimd.dma_start(
                    lhsb[:], _kp(outst_bf[:])[:, :, m * 128:(m + 1) * 128]
                )
                cls_lhs[m] = lhsb
            lhsb = cls_lhs[m]
            nsl = slice(n * NW, (n + 1) * NW)
            ps = psum_m.tile([128, 512], f32, name="pm")
            for k in range(KT):
                nc.tensor.matmul(
                    ps[:, :NW], lhsb[:, k], cls_wtb[:, k, nsl],
                    start=(k == 0), stop=False,
                )
            nc.tensor.matmul(
                ps[:, :NW], ones_bf[:], clsbr[:, nsl], start=False, stop=True
            )
            osb = clsout.tile([128, NW], f32, name="osb")
            nc.vector.tensor_copy(osb[:], ps[:, :NW])
            dst = logits_out[:].rearrange("b (mt st) v -> mt st b v", st=4)
            nc.gpsimd.dma_start(dst[m, :, :, nsl], osb[:])

        # ---- the scan -----------------------------------------------------
        # round r: mm0 -> h0_r slice (step r); mm1 -> h1_{r-1} slice
        # (step r-1); C_r = AllGather(h0_r, h1_{r-1}) slices, f32.
        # lastT for step r-2 also computed in round r.
        cc_out_prev = None
        h_fin_t = [None, None]
        c_fin_t = [None, None]
        cls_emitted = 0

        for r in range(T + 2):
            h0t_prev = h0t_cur
            if r >= 1:
                src = cc_out_prev[:].rearrange(
                    "(k l p) c -> p k l c", k=KT, l=2
                )
                h0t_cur = hbuf.tile([128, KT, B], f32, name="h0t_f")
                nc.sync.dma_start(h0t_cur[:], src[:, :, 0])
                h1t_cur = hbuf.tile([128, KT, B], f32, name="h1t_f")
                nc.scalar.dma_start(h1t_cur[:], src[:, :, 1])
                if r <= T:
                    h0t_b = hbuf.tile([128, KT, B], bf16, name="h0t_b")
                    nc.vector.tensor_copy(h0t_b[:], h0t_cur[:])
                    h1t_b = hbuf.tile([128, KT, B], bf16, name="h1t_b")
                    nc.vector.tensor_copy(h1t_b[:], h1t_cur[:])

            if r <= T:
                cc_in = dram.tile([2 * 128, B], f32, name="cc_in")

            # col-packed scan matmuls: 3 concurrent M=32 col groups
            if r <= T:
                psg = psum_g.tile([128, GW], f32, name="psg")
                for k in range(KT):
                    fl = dict(start=(k == 0), stop=(k == KT - 1),
                              skip_group_check=True)
                    if r < T:
                        nc.tensor.matmul(
                            psg[0:B], h0t_b[:, k], w_sb["w0h"][:, k],
                            tile_position=(0, 0), **fl,
                        )
                    if r >= 1:
                        nc.tensor.matmul(
                            psg[B:2 * B], h0t_b[:, k], w_sb["w1i"][:, k],
                            tile_position=(0, 32), **fl,
                        )
                        nc.tensor.matmul(
                            psg[2 * B:3 * B], h1t_b[:, k], w_sb["w1h"][:, k],
                            tile_position=(0, 64), **fl,
                        )

            if r < T:
                xp0 = st.tile([B, GW], bf16, name="xp0s", bufs=2)
                nc.sync.dma_start(xp0[:], xpj_dram[0, r * B:(r + 1) * B, :])
                h0_new, c0_new = lstm_cell(psg[0:B], xp0[:], c_s[0], "0")
                c_s[0] = c0_new
                if r == T - 1:
                    h_fin_t[0], c_fin_t[0] = h0_new, c0_new
                bt0 = st.tile([B, HSH], f32, name="bt0", bufs=2)
                nc.vector.transpose(bt0[:], h0_new[:])
                bt0_prev = bt0
                nc.gpsimd.dma_start(
                    cc_in[0:128, :].rearrange("(i j) b -> j i b", j=B),
                    bt0[:].rearrange("p (i b) -> p i b", b=B),
                )
            elif r == T:
                nc.gpsimd.dma_start(
                    cc_in[0:128, :].rearrange("(i j) b -> j i b", j=B),
                    bt0_prev[:].rearrange("p (i b) -> p i b", b=B),
                )

            if 1 <= r <= T:
                xp1 = st.tile([B, GW], bf16, name="xp1s", bufs=2)
                nc.sync.dma_start(
                    xp1[:], xpj_dram[1, (r - 1) * B:r * B, :]
                )
                h1_new, c1_new = lstm_cell(
                    psg[B:2 * B], xp1[:], c_s[1], "1",
                    extra_ps=psg[2 * B:3 * B],
                )
                c_s[1] = c1_new
                if r == T:
                    h_fin_t[1], c_fin_t[1] = h1_new, c1_new
                bt1 = st.tile([B, HSH], f32, name="bt1", bufs=2)
                nc.vector.transpose(bt1[:], h1_new[:])
                nc.gpsimd.dma_start(
                    cc_in[128:256, :].rearrange("(i j) b -> j i b", j=B),
                    bt1[:].rearrange("p (i b) -> p i b", b=B),
                )
            elif r == 0:
                nc.gpsimd.dma_start(cc_in[128:256, :], h1isl[:])

            # lastT for step t = r-2 (x_t + h0_t + h1_t), f32 — off the
            # critical path; emitted after the recurrence work on purpose.
            if r >= 2:
                t_step = r - 2
                csl = slice(t_step * B, (t_step + 1) * B)
                xtc = st.tile([128, KT, B], f32, name="xtc")
                nc.scalar.dma_start(xtc[:], _kp(xtr[:])[:, :, csl])
                lastt = st.tile([128, KT, B], f32, name="lastt")
                nc.vector.tensor_add(lastt[:], h0t_prev[:], h1t_cur[:])
                nc.vector.tensor_add(lastt[:], lastt[:], xtc[:])
                nc.scalar.dma_start(_kp(outst[:])[:, :, csl], lastt[:])
                lastb = st.tile([128, KT, B], bf16, name="lastb")
                nc.scalar.copy(lastb[:], lastt[:])
                nc.scalar.dma_start(_kp(outst_bf[:])[:, :, csl], lastb[:])

            if r <= T:
                cc_out = dram.tile([NCORES * 2 * 128, B], f32, name="cc_out")
                nc.gpsimd.collective_compute(
                    "AllGather",
                    ALU.bypass,
                    replica_groups=rg,
                    ins=[cc_in[:].opt()],
                    outs=[cc_out[:].opt()],
                )
                cc_out_prev = cc_out

            budget = 2
            while budget > 0 and cls_emitted < MT * NT:
                m, n = divmod(cls_emitted, NT)
                if r < 4 * m + 5 and r < T + 1:
                    break
                emit_cls_tile(m, n)
                cls_emitted += 1
                budget -= 1
            if r == T + 1:
                while cls_emitted < MT * NT:
                    m, n = divmod(cls_emitted, NT)
                    emit_cls_tile(m, n)
                    cls_emitted += 1

        # ---- final states -------------------------------------------------
        hc_sb = st.tile([B, 2, HSH], f32, name="hc_sb", bufs=1)
        nc.vector.tensor_copy(hc_sb[:, 0], h_fin_t[0][:])
        nc.vector.tensor_copy(hc_sb[:, 1], h_fin_t[1][:])
        nc.sync.dma_start(h_fin[:].rearrange("l b h -> b l h"), hc_sb[:])
        cc_sb = st.tile([B, 2, HSH], f32, name="cc_sb", bufs=1)
        nc.vector.tensor_copy(cc_sb[:, 0], c_fin_t[0][:])
        nc.vector.tensor_copy(cc_sb[:, 1], c_fin_t[1][:])
        nc.sync.dma_start(c_fin[:].rearrange("l b h -> b l h"), cc_sb[:])

        # ---- attention (f32, replicated; host keeps core 0's copy) --------
        wconst_cm.__exit__(None, None, None)

        with tc.tile_pool(name="att", bufs=2) as att:
            outs_all = att.tile([128, KT, T, B], f32, name="outs_all", bufs=1)
            nc.sync.dma_start(
                outs_all[:],
                outst[:].rearrange("(k p) (t b) -> p k t b", p=128, b=B),
            )
            for b in range(B):
                encb = att.tile([128, KT, S], f32, name="encb")
                nc.sync.dma_start(encb[:], _kp(enct[b]))
                pssc = psum_m.tile([128, 512], f32, name="pm")
                for k in range(KT):
                    nc.tensor.matmul(
                        pssc[0:T, 0:S], outs_all[:, k, :, b], encb[:, k],
                        start=(k == 0), stop=(k == KT - 1),
                    )
                nmax = att.tile([T, 1], f32, name="nmax")
                nc.vector.tensor_reduce(
                    nmax[:], pssc[0:T, 0:S], axis=AXL.X, op=ALU.max, negate=True
                )
                probs = att.tile([T, S], f32, name="probs")
                sums = att.tile([T, 1], f32, name="sums")
                nc.scalar.activation(
                    probs[:], pssc[0:T, 0:S], AF.Exp,
                    bias=nmax[:], scale=1.0, accum_out=sums[:],
                )
                rec = att.tile([T, 1], f32, name="rec")
                nc.vector.reciprocal(rec[:], sums[:])
                nc.vector.tensor_scalar_mul(probs[:], probs[:], rec[:])
                pst = psum_m.tile([128, 512], f32, name="pm")
                nc.tensor.transpose(pst[0:S, 0:T], probs[:], ident[0:T, 0:T])
                asb = att.tile([128, T], f32, name="asb")
                nc.vector.tensor_copy(asb[:], pst[0:S, 0:T])
                nc.sync.dma_start(attn_out[b], asb[:])

    nc.compile()
    return nc


def prep_in_maps(inputs):
    dec = np.asarray(inputs["dec_input"])
    h0 = np.asarray(inputs["h0"], np.float32)
    c0 = np.asarray(inputs["c0"], np.float32)
    enc = np.asarray(inputs["encoder_outputs"], np.float32)
    emb = np.asarray(inputs["emb"], np.float32)
    W_ih = np.asarray(inputs["W_ih"], np.float32)
    W_hh = np.asarray(inputs["W_hh"], np.float32)
    b_ih = np.asarray(inputs["b_ih"], np.float32)
    b_hh = np.asarray(inputs["b_hh"], np.float32)
    cls_W = np.asarray(inputs["cls_W"], np.float32)
    cls_b = np.asarray(inputs["cls_b"], np.float32)

    # xT columns ordered c = t*B + b
    idx = dec.T.reshape(-1)
    xt_raw = np.ascontiguousarray(emb[idx].T)
    enct_full = np.ascontiguousarray(enc.transpose(0, 2, 1))

    C = np.ascontiguousarray
    in_maps = []
    for k in range(NCORES):
        hs = slice(HSH * k, HSH * (k + 1))
        rows = np.concatenate([
            np.arange(0 * H, 1 * H)[hs],   # i
            np.arange(1 * H, 2 * H)[hs],   # f
            np.arange(3 * H, 4 * H)[hs],   # o
            np.arange(2 * H, 3 * H)[hs],   # g
        ])
        vs = slice(VS * k, VS * (k + 1))
        in_maps.append({
            "xt_raw": xt_raw,
            "w_ih0t": C(W_ih[0][rows].T),
            "w_hh0t": C(W_hh[0][rows].T),
            "w_ih1t": C(W_ih[1][rows].T),
            "w_hh1t": C(W_hh[1][rows].T),
            "b0_row": C(np.stack([b_ih[0][rows], b_hh[0][rows]])),
            "b1_row": C(np.stack([b_ih[1][rows], b_hh[1][rows]])),
            "h0t_init": C(h0[0].T),
            "h1t_init": C(h0[1].T),
            "h1t_init_sl": C(h0[1][:, hs].T),
            "c_init": C(c0[:, :, hs]),
            "enct": enct_full,
            "cls_wt": C(cls_W[vs].T),
            "cls_b_row": C(cls_b[vs][None, :]),
        })
    return in_maps


def assemble(results):
    logits = np.concatenate([r["logits_shard"] for r in results], axis=2)
    hT = np.concatenate([r["h_final"] for r in results], axis=2)
    cT = np.concatenate([r["c_final"] for r in results], axis=2)
    attn = results[0]["attn_full"]
    return logits, hT, cT, attn


def run_on_hw(nc, in_maps, trace=False):
    from concourse import bass_utils
    return bass_utils.run_bass_kernel_spmd(
        nc, in_maps, core_ids=list(range(NCORES)), trace=trace
    )


def kernel(**inputs):
    in_maps = prep_in_maps(inputs)
    nc = build_nc()
    res = run_on_hw(nc, in_maps, trace=False)
    return assemble(res.results)


# revision 20
# speedup vs baseline: 1.1346x; 1.0280x over previous
"""AttentionDecoder (2-layer residual LSTM + dot attention + vocab classifier)
on 8 Trainium2 NeuronCores.

Sharding:
  - Tensor-parallel shard of the LSTM gate matrices [4H,H] over the 4H dim:
    each core owns a 128-wide h-slice of all four gates, both layers; one
    AllGather of the new (h0, h1) slices (f32) per scan step.
  - Input projections x @ W_ih.T hoisted out of the scan.
  - Classifier [V,H] sharded over V, streamed m-tile-by-m-tile overlapped
    with the scan; attention batch-sharded, computed at the end in f32.
All matmuls bf16 except attention scores (f32); LSTM state stays f32.
"""

import numpy as np
from contextlib import ExitStack

B, T, S, H, V = 32, 64, 128, 1024, 32000
NCORES = 8
KT = H // 128            # 8 k-tiles over the contraction dim
HSH = H // NCORES        # 128-wide h-slice per core
GW = 4 * HSH             # 512 gate width per core; free layout [i|f|o|g]
VS = V // NCORES         # 4000 vocab shard
NT = 8                   # classifier n tiles
NW = VS // NT            # 500
MT = (B * T) // 128      # 16 classifier m tiles
BL = B // NCORES         # 4 local batches for attention


def _kp(ap):
    # [H, w] dram AP -> [128, KT, w] iterated (p, k, w)
    return ap.rearrange("(k p) w -> p k w", p=128)


def build_nc():
    import concourse.bacc as bacc
    import concourse.tile as tile
    from concourse.tile import add_dep_helper
    from concourse import mybir
    from concourse.masks import make_identity
    from concourse._compat import get_trn_type

    f32 = mybir.dt.float32
    bf16 = mybir.dt.bfloat16
    AF = mybir.ActivationFunctionType
    ALU = mybir.AluOpType
    AXL = mybir.AxisListType

    nc = bacc.Bacc(get_trn_type() or "TRN2", target_bir_lowering=False, debug=False)

    # ------------- dram parameters (per-core values via in_maps) -----------
    EI, EO = "ExternalInput", "ExternalOutput"
    xt_raw = nc.dram_tensor("xt_raw", [H, B * T], f32, kind=EI)
    w_ih0t = nc.dram_tensor("w_ih0t", [H, GW], f32, kind=EI)
    w_hh0t = nc.dram_tensor("w_hh0t", [H, GW], f32, kind=EI)
    w_ih1t = nc.dram_tensor("w_ih1t", [H, GW], f32, kind=EI)
    w_hh1t = nc.dram_tensor("w_hh1t", [H, GW], f32, kind=EI)
    b0_row = nc.dram_tensor("b0_row", [2, GW], f32, kind=EI)
    b1_row = nc.dram_tensor("b1_row", [2, GW], f32, kind=EI)
    h0t_init = nc.dram_tensor("h0t_init", [H, B], f32, kind=EI)
    h1t_init = nc.dram_tensor("h1t_init", [H, B], f32, kind=EI)
    h1t_init_sl = nc.dram_tensor("h1t_init_sl", [HSH, B], f32, kind=EI)
    c_init = nc.dram_tensor("c_init", [2, B, HSH], f32, kind=EI)
    enct = nc.dram_tensor("enct", [B, H, S], f32, kind=EI)
    cls_wt = nc.dram_tensor("cls_wt", [H, VS], f32, kind=EI)
    cls_b_row = nc.dram_tensor("cls_b_row", [1, VS], f32, kind=EI)

    logits_out = nc.dram_tensor("logits_shard", [B, T, VS], f32, kind=EO)
    attn_out = nc.dram_tensor("attn_full", [B, S, T], f32, kind=EO)
    h_fin = nc.dram_tensor("h_final", [2, B, HSH], f32, kind=EO)
    c_fin = nc.dram_tensor("c_final", [2, B, HSH], f32, kind=EO)

    # internal dram
    xtr = nc.dram_tensor("xtr", [H, B * T], f32)        # relu(x), transposed
    outst = nc.dram_tensor("outst", [H, B * T], f32)    # 'last'^T, f32
    outst_bf = nc.dram_tensor("outst_bf", [H, B * T], bf16)
    xpj_dram = nc.dram_tensor("xpj_dram", [2, B * T, GW], bf16)

    rg = [list(range(NCORES))]

    with tile.TileContext(nc) as tc, ExitStack() as ctx:
        const = ctx.enter_context(tc.tile_pool(name="const", bufs=1))
        st = ctx.enter_context(tc.tile_pool(name="st", bufs=3))
        hbuf = ctx.enter_context(tc.tile_pool(name="hbuf", bufs=3))
        clsout = ctx.enter_context(tc.tile_pool(name="clsout", bufs=2))
        clsin = ctx.enter_context(tc.tile_pool(name="clsin", bufs=2))
        psum_g = ctx.enter_context(tc.tile_pool(name="psum_g", bufs=4, space="PSUM"))
        psum_m = ctx.enter_context(tc.tile_pool(name="psum_m", bufs=3, space="PSUM"))
        dram = ctx.enter_context(tc.tile_pool(name="dram", bufs=3, space="DRAM"))

        ident = const.tile([128, 128], f32, name="ident")
        make_identity(nc, ident[:])
        ones_row = const.tile([2, 128], f32, name="ones_row")
        nc.gpsimd.memset(ones_row[:], 1.0)
        ones_bf = const.tile([1, 128], bf16, name="ones_bf")
        nc.gpsimd.memset(ones_bf[:], 1.0)
        ident_bf = const.tile([B, B], bf16, name="ident_bf")
        make_identity(nc, ident_bf[:])

        # ---- load scan weights + classifier weights (f32 -> bf16 sbuf) ----
        w_sb = {}
        wconst_cm = tc.tile_pool(name="wconst", bufs=1)
        wconst = wconst_cm.__enter__()
        with tc.tile_pool(name="wload", bufs=2) as wload:
            for name, drt in (("w0h", w_hh0t), ("w1i", w_ih1t), ("w1h", w_hh1t)):
                out = wconst.tile([128, KT, GW], bf16, name=name)
                for k in range(KT):
                    tmp = wload.tile([128, GW], f32, name="wtmp")
                    nc.sync.dma_start(tmp[:], _kp(drt[:])[:, k])
                    nc.vector.tensor_copy(out[:, k], tmp[:])
                w_sb[name] = out
            cls_wtb = wconst.tile([128, KT, VS], bf16, name="cls_wtb")
            for n in range(NT):
                for k in range(KT):
                    tmp = wload.tile([128, NW], f32, name="clstmp")
                    nc.sync.dma_start(
                        tmp[:], _kp(cls_wt[:])[:, k, n * NW:(n + 1) * NW]
                    )
                    nc.vector.tensor_copy(cls_wtb[:, k, n * NW:(n + 1) * NW], tmp[:])
            b0r = const.tile([2, GW], f32, name="b0r")
            nc.sync.dma_start(b0r[:], b0_row[:])
            b1r = const.tile([2, GW], f32, name="b1r")
            nc.sync.dma_start(b1r[:], b1_row[:])
            clsbr = wconst.tile([1, VS], bf16, name="clsbr")
            for n in range(NT):
                clsbr_f = wload.tile([1, NW], f32, name="clsbr_f")
                nc.sync.dma_start(clsbr_f[:], cls_b_row[:, n * NW:(n + 1) * NW])
                nc.vector.tensor_copy(clsbr[:, n * NW:(n + 1) * NW], clsbr_f[:])

        # ---- relu(x) -> xtr, and hoisted input projections ----------------
        with tc.tile_pool(name="xload", bufs=2) as xload:
            for m in range(MT):
                xc = xload.tile([128, KT, 128], f32, name="xstage")
                nc.sync.dma_start(
                    xc[:], _kp(xt_raw[:])[:, :, m * 128:(m + 1) * 128]
                )
                nc.scalar.activation(xc[:], xc[:], AF.Relu)
                nc.sync.dma_start(
                    _kp(xtr[:])[:, :, m * 128:(m + 1) * 128], xc[:]
                )
            for l, wdr in ((0, w_ih0t), (1, w_ih1t)):
                brow = b0r if l == 0 else b1r
                wb = xload.tile([128, KT, GW], bf16, name="wib", bufs=1)
                for k in range(KT):
                    tmp = xload.tile([128, GW], f32, name="wtmp2")
                    nc.sync.dma_start(tmp[:], _kp(wdr[:])[:, k])
                    nc.vector.tensor_copy(wb[:, k], tmp[:])
                for m in range(MT):
                    xmf = xload.tile([128, KT, 128], f32, name="xstage")
                    nc.sync.dma_start(
                        xmf[:], _kp(xtr[:])[:, :, m * 128:(m + 1) * 128]
                    )
                    xmb = xload.tile([128, KT, 128], bf16, name="xmb")
                    nc.vector.tensor_copy(xmb[:], xmf[:])
                    ps = psum_m.tile([128, 512], f32, name="pm")
                    for k in range(KT):
                        nc.tensor.matmul(
                            ps[:, :GW], xmb[:, k], wb[:, k],
                            start=(k == 0), stop=False,
                        )
                    nc.tensor.matmul(
                        ps[:, :GW], ones_row[:], brow[:], start=False, stop=True
                    )
                    xpb = xload.tile([128, GW], bf16, name="xpb")
                    nc.vector.tensor_copy(xpb[:], ps[:, :GW])
                    nc.sync.dma_start(
                        xpj_dram[l, m * 128:(m + 1) * 128, :], xpb[:]
                    )

        # ---- initial state ------------------------------------------------
        h0t_f_init = hbuf.tile([128, KT, B], f32, name="h0t_fi", bufs=1)
        nc.sync.dma_start(h0t_f_init[:], _kp(h0t_init[:]))
        h1t_f_init = hbuf.tile([128, KT, B], f32, name="h1t_fi", bufs=1)
        nc.sync.dma_start(h1t_f_init[:], _kp(h1t_init[:]))
        h0t_b = hbuf.tile([128, KT, B], bf16, name="h0t_b")
        nc.vector.tensor_copy(h0t_b[:], h0t_f_init[:])
        h1t_b = hbuf.tile([128, KT, B], bf16, name="h1t_b")
        nc.vector.tensor_copy(h1t_b[:], h1t_f_init[:])

        c_s = []
        for l in range(2):
            cs = st.tile([B, HSH], f32, name=f"c{l}s")
            nc.sync.dma_start(cs[:], c_init[l])
            c_s.append(cs)

        h1isl_f = st.tile([128, B], f32, name="h1islf", bufs=1)
        nc.sync.dma_start(h1isl_f[:], h1t_init_sl[:])
        h1isl = st.tile([128, B], bf16, name="h1isl", bufs=1)
        nc.vector.tensor_copy(h1isl[:], h1isl_f[:])
        bt0_prev = None

        def lstm_cell(gates_ps, xpj_sl, c_old, lname, extra_ps=None):
            """gates_ps [B,GW] psum; xpj_sl [B,GW] bf16 or None (gates
            complete in psum); free layout [i|f|o|g]*HSH.  extra_ps: second
            psum operand (HW allows one PSUM read per DVE op)."""
            if xpj_sl is None:
                g = gates_ps
            else:
                g = st.tile([B, GW], f32, name=f"g{lname}", bufs=2)
                nc.vector.tensor_add(g[:], gates_ps[:], xpj_sl)
                if extra_ps is not None:
                    nc.vector.tensor_add(g[:], g[:], extra_ps[:])
            sact = st.tile([B, 3 * HSH], f32, name=f"s{lname}", bufs=2)
            nc.scalar.activation(sact[:], g[:, : 3 * HSH], AF.Sigmoid)
            gact = st.tile([B, HSH], f32, name=f"t{lname}", bufs=2)
            nc.scalar.activation(gact[:], g[:, 3 * HSH:], AF.Tanh)
            t1 = st.tile([B, HSH], f32, name=f"t1{lname}", bufs=2)
            nc.vector.tensor_mul(t1[:], sact[:, :HSH], gact[:])
            c_new = st.tile([B, HSH], f32, name=f"c{lname}n")
            nc.vector.tensor_mul(c_new[:], c_old[:], sact[:, HSH:2 * HSH])
            nc.vector.tensor_add(c_new[:], c_new[:], t1[:])
            ct = st.tile([B, HSH], f32, name=f"ct{lname}", bufs=2)
            nc.scalar.activation(ct[:], c_new[:], AF.Tanh)
            h_new = st.tile([B, HSH], f32, name=f"h{lname}n", bufs=2)
            nc.vector.tensor_mul(h_new[:], sact[:, 2 * HSH:], ct[:])
            return h_new, c_new

        cls_lhs = {}

        def emit_cls_tile(m, n, rnd):
            if n == 0:
                lhsb = clsin.tile([128, KT, 128], bf16, name="lhsb")
                nc.sync.dma_start(
                    lhsb[:], _kp(outst_bf[:])[:, :, m * 128:(m + 1) * 128]
                )
                cls_lhs[m] = lhsb
            lhsb = cls_lhs[m]
            nsl = slice(n * NW, (n + 1) * NW)
            ps = psum_m.tile([128, 512], f32, name="pm")
            for k in range(KT):
                nc.tensor.matmul(
                    ps[:, :NW], lhsb[:, k], cls_wtb[:, k, nsl],
                    start=(k == 0), stop=False,
                )
            last_cls_mm[rnd] = nc.tensor.matmul(
                ps[:, :NW], ones_bf[:], clsbr[:, nsl], start=False, stop=True
            )
            osb = clsout.tile([128, NW], f32, name="osb")
            nc.vector.tensor_copy(osb[:], ps[:, :NW])
            dst = logits_out[:].rearrange("b (mt st) v -> mt st b v", st=4)
            nc.gpsimd.dma_start(dst[m, :, :, nsl], osb[:])

        # ---- the scan -----------------------------------------------------
        # round r: mm0 -> h0_r slice (step r); mm1 -> h1_{r-1} slice
        # (step r-1); C_r = AllGather(h0_r, h1_{r-1}) slices, f32.
        # lastT for step r-2 also computed in round r.
        cc_out_prev = None
        h_fin_t = [None, None]
        c_fin_t = [None, None]
        cls_emitted = 0

        first_mm = {}
        last_cls_mm = {}
        for r in range(T + 3):
            h0t_prev = h0t_b
            if 1 <= r <= T + 1:
                src = cc_out_prev[:].rearrange(
                    "(k l p) c -> p k l c", k=KT, l=2
                )
                h0t_b = hbuf.tile([128, KT, B], bf16, name="h0t_b")
                nc.sync.dma_start(h0t_b[:], src[:, :, 0])
                h1t_b = hbuf.tile([128, KT, B], bf16, name="h1t_b")
                nc.scalar.dma_start(h1t_b[:], src[:, :, 1])

            if r <= T:
                cc_in = dram.tile([2 * 128, B], bf16, name="cc_in")

            # col-packed scan matmuls: 3 concurrent M=32 col groups
            if r <= T:
                if r < T:
                    xp0 = st.tile([B, GW], bf16, name="xp0s", bufs=2)
                    nc.sync.dma_start(
                        xp0[:], xpj_dram[0, r * B:(r + 1) * B, :]
                    )
                psg = psum_g.tile([128, GW], f32, name="psg")
                for k in range(KT):
                    fl = dict(start=(k == 0), stop=False,
                              skip_group_check=True)
                    if r < T:
                        i0 = nc.tensor.matmul(
                            psg[0:B], h0t_b[:, k], w_sb["w0h"][:, k],
                            tile_position=(0, 0), **fl,
                        )
                        if k == 0:
                            first_mm[r] = i0
                    if r >= 1:
                        i1 = nc.tensor.matmul(
                            psg[B:2 * B], h0t_b[:, k], w_sb["w1i"][:, k],
                            tile_position=(0, 32),
                            start=(k == 0), stop=(k == KT - 1),
                            skip_group_check=True,
                        )
                        if k == 0 and r == T:
                            first_mm[r] = i1
                        nc.tensor.matmul(
                            psg[2 * B:3 * B], h1t_b[:, k], w_sb["w1h"][:, k],
                            tile_position=(0, 64),
                            start=(k == 0), stop=(k == KT - 1),
                            skip_group_check=True,
                        )
                if r < T:
                    # fold the hoisted input projection into psum on the PE
                    nc.tensor.matmul(
                        psg[0:B], ident_bf[:], xp0[:],
                        tile_position=(0, 0), start=False, stop=True,
                        skip_group_check=True,
                    )
                # order: this round's scan MMs after prev round's cls MMs
                if r - 1 in last_cls_mm and r in first_mm:
                    add_dep_helper(
                        first_mm[r].ins, last_cls_mm[r - 1].ins, sync=False,
                        reason="fill AG window with classifier work",
                    )

            # classifier tiles fill the PE during elementwise + AllGather
            budget = 2
            while budget > 0 and cls_emitted < MT * NT:
                m, n = divmod(cls_emitted, NT)
                if r < 4 * m + 6 and r < T + 2:
                    break
                emit_cls_tile(m, n, r)
                cls_emitted += 1
                budget -= 1
            if r == T + 2:
                while cls_emitted < MT * NT:
                    m, n = divmod(cls_emitted, NT)
                    emit_cls_tile(m, n, r)
                    cls_emitted += 1

            if r < T:
                h0_new, c0_new = lstm_cell(psg[0:B], None, c_s[0], "0")
                c_s[0] = c0_new
                if r == T - 1:
                    h_fin_t[0], c_fin_t[0] = h0_new, c0_new
                h0nb = st.tile([B, HSH], bf16, name="h0nb", bufs=2)
                nc.vector.tensor_copy(h0nb[:], h0_new[:])
                bt0 = st.tile([B, HSH], bf16, name="bt0", bufs=2)
                nc.vector.transpose(bt0[:], h0nb[:])
                bt0_prev = bt0
                nc.gpsimd.dma_start(
                    cc_in[0:128, :].rearrange("(i j) b -> j i b", j=B),
                    bt0[:].rearrange("p (i b) -> p i b", b=B),
                )
            elif r == T:
                nc.gpsimd.dma_start(
                    cc_in[0:128, :].rearrange("(i j) b -> j i b", j=B),
                    bt0_prev[:].rearrange("p (i b) -> p i b", b=B),
                )

            if 1 <= r <= T:
                xp1 = st.tile([B, GW], bf16, name="xp1s", bufs=2)
                nc.sync.dma_start(
                    xp1[:], xpj_dram[1, (r - 1) * B:r * B, :]
                )
                h1_new, c1_new = lstm_cell(
                    psg[B:2 * B], xp1[:], c_s[1], "1",
                    extra_ps=psg[2 * B:3 * B],
                )
                c_s[1] = c1_new
                if r == T:
                    h_fin_t[1], c_fin_t[1] = h1_new, c1_new
                h1nb = st.tile([B, HSH], bf16, name="h1nb", bufs=2)
                nc.vector.tensor_copy(h1nb[:], h1_new[:])
                bt1 = st.tile([B, HSH], bf16, name="bt1", bufs=2)
                nc.vector.transpose(bt1[:], h1nb[:])
                nc.gpsimd.dma_start(
                    cc_in[128:256, :].rearrange("(i j) b -> j i b", j=B),
                    bt1[:].rearrange("p (i b) -> p i b", b=B),
                )
            elif r == 0:
                nc.gpsimd.dma_start(cc_in[128:256, :], h1isl[:])

            # lastT for step t = r-2 (x_t + h0_t + h1_t), f32 — off the
            # critical path; emitted after the recurrence work on purpose.
            if 2 <= r <= T + 1:
                t_step = r - 2
                csl = slice(t_step * B, (t_step + 1) * B)
                xtc = st.tile([128, KT, B], f32, name="xtc")
                nc.scalar.dma_start(xtc[:], _kp(xtr[:])[:, :, csl])
                lastt = st.tile([128, KT, B], f32, name="lastt")
                nc.vector.tensor_add(lastt[:], h0t_prev[:], h1t_b[:])
                nc.vector.tensor_add(lastt[:], lastt[:], xtc[:])
                nc.scalar.dma_start(_kp(outst[:])[:, :, csl], lastt[:])
                lastb = st.tile([128, KT, B], bf16, name="lastb")
                nc.scalar.copy(lastb[:], lastt[:])
                nc.scalar.dma_start(_kp(outst_bf[:])[:, :, csl], lastb[:])

            if r <= T:
                cc_out = dram.tile([NCORES * 2 * 128, B], bf16, name="cc_out")
                nc.gpsimd.collective_compute(
                    "AllGather",
                    ALU.bypass,
                    replica_groups=rg,
                    ins=[cc_in[:].opt()],
                    outs=[cc_out[:].opt()],
                )
                cc_out_prev = cc_out


        # ---- final states -------------------------------------------------
        hc_sb = st.tile([B, 2, HSH], f32, name="hc_sb", bufs=1)
        nc.vector.tensor_copy(hc_sb[:, 0], h_fin_t[0][:])
        nc.vector.tensor_copy(hc_sb[:, 1], h_fin_t[1][:])
        nc.sync.dma_start(h_fin[:].rearrange("l b h -> b l h"), hc_sb[:])
        cc_sb = st.tile([B, 2, HSH], f32, name="cc_sb", bufs=1)
        nc.vector.tensor_copy(cc_sb[:, 0], c_fin_t[0][:])
        nc.vector.tensor_copy(cc_sb[:, 1], c_fin_t[1][:])
        nc.sync.dma_start(c_fin[:].rearrange("l b h -> b l h"), cc_sb[:])

        # ---- attention (f32, replicated; host keeps core 0's copy) --------
        wconst_cm.__exit__(None, None, None)

        with tc.tile_pool(name="att", bufs=2) as att:
            outs_all = att.tile([128, KT, T, B], f32, name="outs_all", bufs=1)
            nc.sync.dma_start(
                outs_all[:],
                outst[:].rearrange("(k p) (t b) -> p k t b", p=128, b=B),
            )
            for b in range(B):
                encb = att.tile([128, KT, S], f32, name="encb")
                nc.sync.dma_start(encb[:], _kp(enct[b]))
                pssc = psum_m.tile([128, 512], f32, name="pm")
                for k in range(KT):
                    nc.tensor.matmul(
                        pssc[0:T, 0:S], outs_all[:, k, :, b], encb[:, k],
                        start=(k == 0), stop=(k == KT - 1),
                    )
                nmax = att.tile([T, 1], f32, name="nmax")
                nc.vector.tensor_reduce(
                    nmax[:], pssc[0:T, 0:S], axis=AXL.X, op=ALU.max, negate=True
                )
                probs = att.tile([T, S], f32, name="probs")
                sums = att.tile([T, 1], f32, name="sums")
                nc.scalar.activation(
                    probs[:], pssc[0:T, 0:S], AF.Exp,
                    bias=nmax[:], scale=1.0, accum_out=sums[:],
                )
                rec = att.tile([T, 1], f32, name="rec")
                nc.vector.reciprocal(rec[:], sums[:])
                nc.vector.tensor_scalar_mul(probs[:], probs[:], rec[:])
                pst = psum_m.tile([128, 512], f32, name="pm")
                nc.tensor.transpose(pst[0:S, 0:T], probs[:], ident[0:T, 0:T])
                asb = att.tile([128, T], f32, name="asb")
                nc.vector.tensor_copy(asb[:], pst[0:S, 0:T])
                nc.sync.dma_start(attn_out[b], asb[:])

    nc.compile()
    return nc


def prep_in_maps(inputs):
    dec = np.asarray(inputs["dec_input"])
    h0 = np.asarray(inputs["h0"], np.float32)
    c0 = np.asarray(inputs["c0"], np.float32)
    enc = np.asarray(inputs["encoder_outputs"], np.float32)
    emb = np.asarray(inputs["emb"], np.float32)
    W_ih = np.asarray(inputs["W_ih"], np.float32)
    W_hh = np.asarray(inputs["W_hh"], np.float32)
    b_ih = np.asarray(inputs["b_ih"], np.float32)
    b_hh = np.asarray(inputs["b_hh"], np.float32)
    cls_W = np.asarray(inputs["cls_W"], np.float32)
    cls_b = np.asarray(inputs["cls_b"], np.float32)

    # xT columns ordered c = t*B + b
    idx = dec.T.reshape(-1)
    xt_raw = np.ascontiguousarray(emb[idx].T)
    enct_full = np.ascontiguousarray(enc.transpose(0, 2, 1))

    C = np.ascontiguousarray
    in_maps = []
    for k in range(NCORES):
        hs = slice(HSH * k, HSH * (k + 1))
        rows = np.concatenate([
            np.arange(0 * H, 1 * H)[hs],   # i
            np.arange(1 * H, 2 * H)[hs],   # f
            np.arange(3 * H, 4 * H)[hs],   # o
            np.arange(2 * H, 3 * H)[hs],   # g
        ])
        vs = slice(VS * k, VS * (k + 1))
        in_maps.append({
            "xt_raw": xt_raw,
            "w_ih0t": C(W_ih[0][rows].T),
            "w_hh0t": C(W_hh[0][rows].T),
            "w_ih1t": C(W_ih[1][rows].T),
            "w_hh1t": C(W_hh[1][rows].T),
            "b0_row": C(np.stack([b_ih[0][rows], b_hh[0][rows]])),
            "b1_row": C(np.stack([b_ih[1][rows], b_hh[1][rows]])),
            "h0t_init": C(h0[0].T),
            "h1t_init": C(h0[1].T),
            "h1t_init_sl": C(h0[1][:, hs].T),
            "c_init": C(c0[:, :, hs]),
            "enct": enct_full,
            "cls_wt": C(cls_W[vs].T),
            "cls_b_row": C(cls_b[vs][None, :]),
        })
    return in_maps


def assemble(results):
    logits = np.concatenate([r["logits_shard"] for r in results], axis=2)
    hT = np.concatenate([r["h_final"] for r in results], axis=2)
    cT = np.concatenate([r["c_final"] for r in results], axis=2)
    attn = results[0]["attn_full"]
    return logits, hT, cT, attn


def run_on_hw(nc, in_maps, trace=False):
    from concourse import bass_utils
    return bass_utils.run_bass_kernel_spmd(
        nc, in_maps, core_ids=list(range(NCORES)), trace=trace
    )


def kernel(**inputs):
    in_maps = prep_in_maps(inputs)
    nc = build_nc()
    res = run_on_hw(nc, in_maps, trace=False)
    return assemble(res.results)
